# revision 24
# baseline (speedup 1.0000x reference)
"""Trainium2 Bass kernel for nn_Decoder (attention GRU decoder + classifier).

Key algebraic simplification: the additive-attention logits are
  s[b,t] = score_x[b,t] + (h @ Wa_h)[b]
and softmax over t is invariant to the per-b constant shift, so the attention
weights -- and therefore ctx and gi = ctx @ W_ih.T -- are the same for all
n_steps steps.  The recurrence reduces to gh = h @ W_hh.T per step.

Sharding: pure data-parallel over batch, 16 rows per core, no collectives.

Perf structure (vs the first-generation kernel):
- All weight transposes happen on HOST (numpy) -- no PE transpose matmuls.
- Attention scores via fused DVE tensor_tensor_reduce (mult + free-dim accum).
- ctx via m=1 n=512 matmuls (4 per batch row) instead of 16 n=1 matmuls.
- gi/bias folded into the gate PSUM accumulation with a single identity16
  matmul per gate (no separate 16x16x512 + 1x16x512 bias matmuls).
- GRU chain in fp16 SBUF (DVE 2x mode), gate order N,R,Z to hide the tail.
- Classifier tiles interleaved into the recurrence's PE gaps (keeps the PE
  p-state at full clock and removes most of the classifier tail).
- Bias adds are compiled only when the corresponding input is nonzero.
"""

import sys

for _p in ("/root/.axon_site",):
    if _p not in sys.path:
        sys.path.insert(0, _p)

import numpy as np

import concourse.bass as bass
import concourse.bacc as bacc
import concourse.mybir as mybir
from concourse import bass_isa, tile
from concourse.bass_utils import run_bass_kernel_spmd

dt = mybir.dt
AF = mybir.ActivationFunctionType
ALU = mybir.AluOpType

N_CORES = 8
B, T, D, H, C = 128, 512, 512, 512, 4367
BL = B // N_CORES  # 16 batch rows per core
TC, DC, HC = T // 128, D // 128, H // 128
G3 = 3 * H  # 1536

F16 = dt.float16


def _build(n_steps, nz):
    """nz: dict of which biases are nonzero (b_ih, b_hh, b_proj, b_cls)."""
    S = n_steps
    nc = bacc.Bacc("TRN2", target_bir_lowering=False, debug=False,
                   num_devices=N_CORES)

    x_d = nc.dram_tensor("x", [BL, T, D], F16, kind="ExternalInput").ap()
    xlast_d = nc.dram_tensor("xlast", [BL, D], F16, kind="ExternalInput").ap()
    wax_d = nc.dram_tensor("wax_b", [128, D], F16, kind="ExternalInput").ap()
    wihT_d = nc.dram_tensor("wihT", [D, G3], F16, kind="ExternalInput").ap()
    whhT_d = nc.dram_tensor("whhT", [H, G3], F16, kind="ExternalInput").ap()
    wprojT_d = nc.dram_tensor("wprojT", [D, H], F16, kind="ExternalInput").ap()
    wclsT_d = nc.dram_tensor("wclsT", [H, C], F16, kind="ExternalInput").ap()
    consts_d = nc.dram_tensor("consts", [128, 160], F16, kind="ExternalInput").ap()
    bias_d = {}
    if nz["b_ih"] or nz["b_hh"]:
        # combined per-gate additive rows: row0..2 = (b_ih+b_hh) for r,z; b_ih for n
        # row3 = b_hh for n (goes inside the r* term)
        bias_d["gates"] = nc.dram_tensor(
            "bias_gates", [4, H], dt.float32, kind="ExternalInput").ap()
    if nz["b_proj"]:
        bias_d["proj"] = nc.dram_tensor(
            "bias_proj", [1, H], dt.float32, kind="ExternalInput").ap()
    if nz["b_cls"]:
        bias_d["cls"] = nc.dram_tensor(
            "bias_cls", [1, C], dt.float32, kind="ExternalInput").ap()
    y_d = nc.dram_tensor("y", [S, BL, C], F16, kind="ExternalOutput").ap()
    y_flat = y_d.rearrange("s b c -> (s b) c")

    with tile.TileContext(nc) as tc:
        _emit(nc, tc, S, nz, x_d, xlast_d, wax_d, wihT_d, whhT_d, wprojT_d,
              wclsT_d, consts_d, bias_d, y_flat)
    nc.compile()
    return nc


def _emit(nc, tc, S, nz, x_d, xlast_d, wax_d, wihT_d, whhT_d, wprojT_d,
          wclsT_d, consts_d, bias_d, y_flat):
    from contextlib import ExitStack
    ctx_stack = ExitStack()
    with ctx_stack:
        wts = ctx_stack.enter_context(tc.tile_pool(name="wts", bufs=1))
        xp = ctx_stack.enter_context(tc.tile_pool(name="xp", bufs=BL))
        work = ctx_stack.enter_context(tc.tile_pool(name="work", bufs=2))
        ps_g = ctx_stack.enter_context(
            tc.tile_pool(name="ps_g", bufs=1, space="PSUM"))
        ps_cls = ctx_stack.enter_context(
            tc.tile_pool(name="ps_cls", bufs=3, space="PSUM"))
        ps_tr = ctx_stack.enter_context(
            tc.tile_pool(name="ps_tr", bufs=1, space="PSUM"))
        ps_misc = ctx_stack.enter_context(
            tc.tile_pool(name="ps_misc", bufs=1, space="PSUM"))

        # ---- constants / weights (host-precomputed layouts) ----
        consts = wts.tile([128, 160], F16)
        nc.sync.dma_start(consts[:], consts_d)
        ident16 = consts[:16, :16]        # I16
        ones_col = consts[:, 16:17]       # [128,1] ones
        ones_row16 = consts[:1, 17:33]    # [1,16] ones
        ones_row128 = consts[:1, 17:145]  # [1,128] ones

        wax = wts.tile([128, D], F16)
        nc.sync.dma_start(wax[:], wax_d)
        wihT = wts.tile([128, DC, G3], F16)
        nc.sync.dma_start(wihT[:], wihT_d.rearrange("(c p) n -> p c n", p=128))
        wprojT = wts.tile([128, DC, H], F16)
        nc.sync.dma_start(wprojT[:], wprojT_d.rearrange("(c p) n -> p c n", p=128))
        xlast = wts.tile([BL, D], F16)
        nc.sync.dma_start(xlast[:], xlast_d)

        bias_t = {}
        if "gates" in bias_d:
            bias_t["gates"] = wts.tile([4, H], F16)
            nc.sync.dma_start(bias_t["gates"][:], bias_d["gates"])
        if "proj" in bias_d:
            bias_t["proj"] = wts.tile([1, H], F16)
            nc.sync.dma_start(bias_t["proj"][:], bias_d["proj"])
        if "cls" in bias_d:
            bias_t["cls"] = wts.tile([1, C], F16)
            nc.sync.dma_start(bias_t["cls"][:], bias_d["cls"])

        # ---- attention: scores on DVE, exp per-b, ctx on PE (interleaved) ----
        s_all = wts.tile([128, BL, TC], dt.float32)
        e_all = wts.tile([128, BL, TC], F16)
        ctx_acc = ps_misc.tile([128, DC, BL], dt.float32, tag="ctxT")
        xb_tiles = []
        for b in range(BL):
            xb = xp.tile([128, TC, D], F16, tag="xb")
            nc.gpsimd.dma_start(
                xb[:], x_d[b].rearrange("(tc tp) d -> tp tc d", tp=128))
            xb_tiles.append(xb)
            junk = work.tile([128, D], F16, tag="junk")
            for tc_i in range(TC):
                nc.vector.scalar_tensor_tensor(
                    out=junk[:], in0=xb[:, tc_i, :], scalar=1.0, in1=wax[:],
                    op0=ALU.mult, op1=ALU.mult,
                    accum_out=s_all[:, b, tc_i:tc_i + 1])
            nc.scalar.activation(e_all[:, b, :], s_all[:, b, :], AF.Exp)
            if b == 0:
                # late-phase weights ride the ACT instruction stream so their
                # DMA issue is *actually* delayed past the x transfers (a
                # dma_start on an idle engine hits the queues immediately,
                # regardless of emission position)
                whhT = wts.tile([128, DC, G3], F16)
                nc.scalar.dma_start(
                    whhT[:], whhT_d.rearrange("(c p) n -> p c n", p=128))
            if b == 11:
                wclsT = wts.tile([128, HC, C], F16)
                nc.scalar.dma_start(
                    wclsT[:], wclsT_d.rearrange("(c p) n -> p c n", p=128))
            # ctx for row b immediately (PE overlaps later rows' scores).
            # One accumulation group per bank across the phase: only the very
            # first matmul carries start=True (it zeroes the whole bank).
            for dc_i in range(DC):
                for tc_i in range(TC):
                    nc.tensor.matmul(
                        ctx_acc[:, dc_i, b:b + 1],
                        xb[:, tc_i, dc_i * 128:(dc_i + 1) * 128],
                        e_all[:, b, tc_i:tc_i + 1],
                        start=(b == 0 and dc_i == 0 and tc_i == 0),
                        stop=(b == BL - 1 and dc_i == DC - 1
                              and tc_i == TC - 1))


        # partition-replicated softmax sums: V-reduce over tc, then a GpSimd
        # all-reduce across partitions (t) -- no PSUM bank needed
        part_sums = wts.tile([128, BL], dt.float32)
        nc.vector.tensor_reduce(out=part_sums[:], in_=e_all[:],
                                axis=mybir.AxisListType.X, op=ALU.add)
        sums_bc = wts.tile([128, BL], dt.float32)
        nc.gpsimd.partition_all_reduce(sums_bc[:], part_sums[:], channels=128,
                                       reduce_op=bass_isa.ReduceOp.add)
        recip_bc = wts.tile([128, BL], dt.float32)
        nc.vector.reciprocal(recip_bc[:], sums_bc[:])

        ctxT = wts.tile([128, DC, BL], F16)
        for dc_i in range(DC):
            nc.vector.tensor_tensor(out=ctxT[:, dc_i, :],
                                    in0=ctx_acc[:, dc_i, :],
                                    in1=recip_bc[:], op=ALU.mult)

        # gi_g = ctx @ W_ih.T (+ biases) ; kept in SBUF fp16 as matmul rhs
        gi = []
        gtags = ["gn", "gr", "gz"]  # allocation tags (shared with gates)
        for g in range(3):
            pg = ps_g.tile([BL, H], dt.float32, tag=gtags[g])
            has_bias = "gates" in bias_t
            for dc_i in range(DC):
                nc.tensor.matmul(pg[:], ctxT[:, dc_i, :],
                                 wihT[:, dc_i, g * H:(g + 1) * H],
                                 start=(dc_i == 0),
                                 stop=(dc_i == DC - 1 and not has_bias))
            if has_bias:
                nc.tensor.matmul(pg[:], ones_row16,
                                 bias_t["gates"][g:g + 1, :],
                                 start=False, stop=True)
            gt = wts.tile([BL, H], F16, tag=f"gi{g}")
            nc.vector.tensor_copy(gt[:], pg[:])
            gi.append(gt)

        # h0 = x_last @ W_proj.T (+ b_proj)
        xlT = work.tile([128, DC, BL], F16, tag="xlT")
        ptx = ps_tr.tile([128, DC, BL], F16, tag="tr")
        for dc_i in range(DC):
            nc.tensor.matmul(ptx[:, dc_i, :],
                             xlast[:, dc_i * 128:(dc_i + 1) * 128],
                             ident16, is_transpose=True,
                             start=(dc_i == 0), stop=(dc_i == DC - 1),
                             skip_group_check=True)
        nc.vector.tensor_copy(xlT[:], ptx[:])
        ph = ps_g.tile([BL, H], dt.float32, tag="gr")
        has_pb = "proj" in bias_t
        for dc_i in range(DC):
            nc.tensor.matmul(ph[:], xlT[:, dc_i, :], wprojT[:, dc_i, :],
                             start=(dc_i == 0),
                             stop=(dc_i == DC - 1 and not has_pb))
        if has_pb:
            nc.tensor.matmul(ph[:], ones_row16, bias_t["proj"][:],
                             start=False, stop=True)
        h_prev = work.tile([BL, H], F16, tag="h")
        nc.vector.tensor_copy(h_prev[:], ph[:])

        # hsT[:, hc, slot*16 + b] ; slot 0 = h0, slot s+1 = step s output
        hsT = wts.tile([128, HC, BL * (S + 1)], F16)

        def transpose_h(h_t, slot):
            # 4 transposes into one PSUM bank (only the first carries
            # start=True, which zeroes the whole bank), then ONE V copy
            pt = ps_tr.tile([128, HC, BL], F16, tag="tr")
            for hc_i in range(HC):
                nc.tensor.matmul(pt[:, hc_i, :],
                                 h_t[:, hc_i * 128:(hc_i + 1) * 128],
                                 ident16, is_transpose=True,
                                 start=(hc_i == 0), stop=(hc_i == HC - 1),
                                 skip_group_check=True)
            nc.vector.tensor_copy(
                hsT[:, :, slot * BL:(slot + 1) * BL], pt[:])

        transpose_h(h_prev, 0)

        # ---- classifier tile machinery (interleaved into the recurrence) ----
        NROW = BL * S  # 352 output rows, s-major
        m_chunks = []
        m0 = 0
        while m0 < NROW:
            m_chunks.append((m0, min(128, NROW - m0)))
            m0 += 128
        n_starts = list(range(0, C, 512))
        # tile (mi, n0) ready after recurrence step: last step covered by chunk
        all_tiles = []
        for mi, (m0, mc_sz) in enumerate(m_chunks):
            ready = (m0 + mc_sz - 1) // BL  # step index (0-based)
            for n0 in n_starts:
                all_tiles.append((ready, mi, n0))

        def emit_cls_matmuls(mi, n0):
            m0, mc_sz = m_chunks[mi]
            n_sz = min(512, C - n0)
            has_cb = "cls" in bias_t
            pt = ps_cls.tile([128, 512], dt.float32, tag="cls")
            for kc in range(HC):
                nc.tensor.matmul(pt[:mc_sz, :n_sz],
                                 hsT[:, kc, BL + m0: BL + m0 + mc_sz],
                                 wclsT[:, kc, n0:n0 + n_sz],
                                 start=(kc == 0),
                                 stop=(kc == HC - 1 and not has_cb))
            if has_cb:
                nc.tensor.matmul(pt[:mc_sz, :n_sz], ones_row128[:, :mc_sz],
                                 bias_t["cls"][:, n0:n0 + n_sz],
                                 start=False, stop=True)
            return pt

        def emit_cls_copyout(pt, mi, n0):
            m0, mc_sz = m_chunks[mi]
            n_sz = min(512, C - n0)
            ot = work.tile([128, 512], F16, tag="cot")
            nc.vector.tensor_copy(ot[:mc_sz, :n_sz], pt[:mc_sz, :n_sz])
            nc.gpsimd.dma_start(y_flat[m0:m0 + mc_sz, n0:n0 + n_sz],
                                ot[:mc_sz, :n_sz])

        pending = list(all_tiles)

        # ---- recurrence ----
        for s in range(S):
            hT = hsT[:, :, s * BL:(s + 1) * BL]
            pg = {}
            for g in (2, 0, 1):  # N first, then R, Z
                tag = gtags[g]
                p = ps_g.tile([BL, H], dt.float32, tag=tag)
                pg[g] = p
                aug_gi = g < 2  # r,z: add gi inside the sigmoid
                nbias = (g == 2 and "gates" in bias_t)
                closer = aug_gi or nbias
                for dc_i in range(DC):
                    nc.tensor.matmul(p[:], hT[:, dc_i, :],
                                     whhT[:, dc_i, g * H:(g + 1) * H],
                                     start=(dc_i == 0),
                                     stop=(dc_i == DC - 1 and not closer))
                if aug_gi:
                    nc.tensor.matmul(p[:], ident16, gi[g][:],
                                     start=False, stop=True)
                elif nbias:
                    nc.tensor.matmul(p[:], ones_row16,
                                     bias_t["gates"][3:4, :],
                                     start=False, stop=True)
            # chain (fp16, SBUF): n-path on the critical path; z early so the
            # ACT engine does ghn, r, z, tanh in that order (z off-path)
            ghn = work.tile([BL, H], F16, tag="ghn")
            nc.scalar.copy(ghn[:], pg[2][:])
            r_t = work.tile([BL, H], F16, tag="r")
            nc.scalar.activation(r_t[:], pg[0][:], AF.Sigmoid)
            z_t = work.tile([BL, H], F16, tag="z")
            nc.scalar.activation(z_t[:], pg[1][:], AF.Sigmoid)
            rhn = work.tile([BL, H], F16, tag="rhn")
            nc.vector.tensor_tensor(out=rhn[:], in0=r_t[:], in1=ghn[:],
                                    op=ALU.mult)
            pre_n = work.tile([BL, H], F16, tag="pre")
            nc.vector.tensor_tensor(out=pre_n[:], in0=rhn[:], in1=gi[2][:],
                                    op=ALU.add)
            n_t = work.tile([BL, H], F16, tag="n")
            nc.scalar.activation(n_t[:], pre_n[:], AF.Tanh)
            # h_new = n + z*(h - n); zh-side precomputed off the tanh path
            hmn = work.tile([BL, H], F16, tag="hmn")
            nc.vector.tensor_tensor(out=hmn[:], in0=h_prev[:], in1=n_t[:],
                                    op=ALU.subtract)
            znm = work.tile([BL, H], F16, tag="znm")
            nc.vector.tensor_tensor(out=znm[:], in0=z_t[:], in1=hmn[:],
                                    op=ALU.mult)
            h_new = work.tile([BL, H], F16, tag="h")
            nc.vector.tensor_tensor(out=h_new[:], in0=n_t[:], in1=znm[:],
                                    op=ALU.add)

            # PE filler during the chain: ready classifier tiles (else dummies)
            budget = 2 if s < S - 1 else 0
            emitted = []
            while pending and pending[0][0] < s and len(emitted) < budget:
                _, mi, n0 = pending.pop(0)
                emitted.append((emit_cls_matmuls(mi, n0), mi, n0))
            if not emitted and s < S - 1:
                # dummy matmuls to keep the PE p-state hot during the chain
                dp = ps_cls.tile([128, 512], dt.float32, tag="cls")
                for _ in range(6):
                    nc.tensor.matmul(dp[:1, :], ones_col[:1, :1], wax[:1, :],
                                     start=True, stop=True)

            transpose_h(h_new, s + 1)
            # cls copy-outs AFTER the transpose copy so V never delays the
            # next step's gate matmuls (V runs its queue in emission order)
            for pt, mi, n0 in emitted:
                emit_cls_copyout(pt, mi, n0)
            h_prev = h_new

        # ---- classifier tail ----
        while pending:
            _, mi, n0 = pending.pop(0)
            pt = emit_cls_matmuls(mi, n0)
            emit_cls_copyout(pt, mi, n0)


_NC_CACHE = {}


def _get_nc(n_steps, nz_key):
    key = (n_steps, nz_key)
    if key not in _NC_CACHE:
        nz = dict(zip(("b_ih", "b_hh", "b_proj", "b_cls"), nz_key))
        _NC_CACHE[key] = _build(n_steps, nz)
    return _NC_CACHE[key]


def _host_prep(inputs):
    x = np.ascontiguousarray(np.asarray(inputs["x"]), dtype=np.float16)
    n_steps = int(np.asarray(inputs["n_steps"]))
    assert x.shape == (B, T, D)

    f16 = lambda a: np.ascontiguousarray(np.asarray(a), dtype=np.float16)
    f32 = lambda a: np.ascontiguousarray(np.asarray(a), dtype=np.float32)

    w = {
        "wihT": f16(np.asarray(inputs["W_ih"], dtype=np.float32).T),
        "whhT": f16(np.asarray(inputs["W_hh"], dtype=np.float32).T),
        "wprojT": f16(np.asarray(inputs["W_proj"], dtype=np.float32).T),
        "wclsT": f16(np.asarray(inputs["W_cls"], dtype=np.float32).T),
    }
    wax_b = np.broadcast_to(
        np.asarray(inputs["W_align"], dtype=np.float32)[0, :D], (128, D))
    w["wax_b"] = f16(wax_b)

    consts = np.zeros((128, 160), dtype=np.float16)
    consts[:16, :16] = np.eye(16, dtype=np.float16)
    consts[:, 16] = 1.0
    consts[0, 17:145] = 1.0
    w["consts"] = consts

    b_ih = f32(inputs["b_ih"])
    b_hh = f32(inputs["b_hh"])
    b_proj = f32(inputs["b_proj"])
    b_cls = f32(inputs["b_cls"])
    nz = {
        "b_ih": bool(np.any(b_ih)), "b_hh": bool(np.any(b_hh)),
        "b_proj": bool(np.any(b_proj)), "b_cls": bool(np.any(b_cls)),
    }
    if nz["b_ih"] or nz["b_hh"]:
        gates = np.zeros((4, H), dtype=np.float32)
        gates[0] = b_ih[:H] + b_hh[:H]              # r
        gates[1] = b_ih[H:2 * H] + b_hh[H:2 * H]    # z
        gates[2] = b_ih[2 * H:]                      # n (i-part)
        gates[3] = b_hh[2 * H:]                      # n (h-part, inside r*)
        w["bias_gates"] = gates
    if nz["b_proj"]:
        w["bias_proj"] = b_proj.reshape(1, H)
    if nz["b_cls"]:
        w["bias_cls"] = b_cls.reshape(1, C)
    # b_align shifts every logit equally -> softmax-invariant, unused.
    return x, n_steps, w, nz


def kernel(**inputs):
    x, n_steps, w, nz = _host_prep(inputs)
    nz_key = tuple(nz[k] for k in ("b_ih", "b_hh", "b_proj", "b_cls"))
    nc = _get_nc(n_steps, nz_key)

    in_maps = []
    for i in range(N_CORES):
        m = dict(w)
        xs = x[i * BL:(i + 1) * BL]
        m["x"] = xs
        m["xlast"] = np.ascontiguousarray(xs[:, T - 1, :])
        in_maps.append(m)
    res = run_bass_kernel_spmd(nc, in_maps, list(range(N_CORES)))
    # y per core: [S, BL, C] -> [BL, S, C]
    out = np.concatenate(
        [np.transpose(res.results[i]["y"], (1, 0, 2)) for i in range(N_CORES)],
        axis=0)
    return out.astype(np.float32)


if __name__ == "__main__":
    rng = np.random.default_rng(0)
    ins = {
        "x": rng.standard_normal((B, T, D)).astype(np.float32),
        "W_proj": (rng.standard_normal((H, D)) * 0.02).astype(np.float32),
        "b_proj": np.zeros(H, np.float32),
        "W_align": (rng.standard_normal((1, H + D)) * 0.02).astype(np.float32),
        "b_align": np.zeros(1, np.float32),
        "W_ih": (rng.standard_normal((G3, D)) * 0.02).astype(np.float32),
        "b_ih": np.zeros(G3, np.float32),
        "W_hh": (rng.standard_normal((G3, H)) * 0.02).astype(np.float32),
        "b_hh": np.zeros(G3, np.float32),
        "W_cls": (rng.standard_normal((C, H)) * 0.02).astype(np.float32),
        "b_cls": np.zeros(C, np.float32),
        "n_steps": np.int64(22),
    }
    y = kernel(**ins)
    print("out", y.shape, y.dtype, float(np.abs(y).max()))
